# revision 20
# baseline (speedup 1.0000x reference)
"""BiDAF attention Trainium2 kernel (fp16 I/O rewrite).

Full-input contract: kernel(**inputs) takes the unsharded tensors
(context [16,2048,128] f32, query [16,128,128] f32, W [384] f32,
context_mask [16,2048] i32, query_mask [16,128] i32) and returns
G = [16, 2048, 512] f32.

Sharding: data-parallel over batch across 8 NeuronCores (2 batches/core).

Key structure (per batch, on device):
  eT[q, c]   = exp(s_cq + s_q + qbias)          (s_c factors out of the
                                                 softmax over q)
  c2q        = eT^T @ query / Z,  Z = col-sums of eT
  m~[c]      = max_q eT[q, c]  (PE transpose + DVE rowmax)
  e_b[c]     = m~ * exp(s_c + cbias)            (exp(max_q S) = m~*exp(s_c))
  q2c        = sum_c e_b * ctx / sum_c e_b
  G blocks: [ctx | c2q | ctx*c2q | ctx*q2c]; block 0 is a pure input
  passthrough filled on the host; the device computes+stores only
  cols 128:512 in fp16 (tolerance 2e-2 >> fp16 rounding).

Host packs fp16 inputs so every DMA descriptor is >= 512B (full DMA bw).
"""

import sys

sys.path.insert(0, "/opt/trn_rl_repo")

import numpy as np

import concourse.bass as bass
import concourse.tile as tile
from concourse import mybir
from concourse.masks import make_identity
from concourse.vector_clock import ScopedClock

B, C_LEN, Q_LEN, H = 16, 2048, 128, 128
N_CORES = 8
B_PER_CORE = B // N_CORES          # 2
N_CTILES = C_LEN // 128            # 16
NGRP = 4                           # tile groups per batch
GS = N_CTILES // NGRP              # tiles per group (4)
F32 = mybir.dt.float32
F16 = mybir.dt.float16
I32 = mybir.dt.int32

MASK_NEG = -30000.0                # fp16-exact, exp() underflows to 0

# fp16 blob columns (per batch): [qT | q | qbias | cbias | w_c w_q w_cq]
BC_QT = 0          # 0:128    queryT [h, q]
BC_Q = 128         # 128:256  query  [q, h]
BC_QB = 256        # 256      qbias col [q, 1]
BC_CB = 257        # 257:273  cbias [p, 16]
BC_WC = 273        # w_c col
BC_WQ = 274        # w_q col
BC_WCQ = 275       # w_cq col
BLOB_COLS = 280    # padded to 560B per partition

MAX_WAITS_PER_INST = 1


def _split_excess_waits(nc, insts):
    """Hoist all but one sync wait per instruction onto preceding nops.

    The walrus build in this container rejects >1 sync wait on an
    instruction's descriptor, while Tile's sem assignment freely attaches
    several. A nop on the same engine right before the instruction stalls
    the engine identically.
    """
    out = []
    for inst in insts:
        si = getattr(inst, "sync_info", None)
        waits = list(si.on_wait) if si is not None and si.on_wait else []
        if len(waits) > MAX_WAITS_PER_INST and type(inst).__name__.startswith("Inst"):
            extra = waits[: -MAX_WAITS_PER_INST or None]
            keep = waits[-MAX_WAITS_PER_INST:]
            for i in range(0, len(extra), MAX_WAITS_PER_INST):
                out.append(
                    mybir.InstNoOp(
                        name=nc.get_next_instruction_name(),
                        sync_info=mybir.SyncInfo(
                            on_wait=extra[i : i + MAX_WAITS_PER_INST], on_update=[]
                        ),
                        bass_nofuse=True,
                        engine=inst.engine,
                    )
                )
            inst.sync_info = mybir.SyncInfo(
                on_wait=keep, on_update=list(si.on_update or [])
            )
        out.append(inst)
    return out


class SplitDrainTileContext(tile.TileContext):
    """TileContext whose tail drain splits its sem waits across SP nops."""

    def _lower_ordered_insts(self, ordered):
        for bb_name in list(ordered.keys()):
            ordered[bb_name] = _split_excess_waits(self.nc, ordered[bb_name])
        return super()._lower_ordered_insts(ordered)

    def _drain_and_barrier(self, tick_clock, wait_clock):
        nc = self.nc
        drain_inst = nc.sync.drain()
        wait_clock.add_sem_waits(
            drain_inst.ins, ScopedClock({None: tick_clock.global_clock})
        )
        si = drain_inst.ins.sync_info
        waits = list(si.on_wait) if si is not None and si.on_wait else []
        if waits:
            drain_inst.ins.sync_info = mybir.SyncInfo(
                on_wait=[], on_update=list(si.on_update or [])
            )
            engs = [nc.sync, nc.vector, nc.scalar, nc.tensor, nc.gpsimd]
            for j, i in enumerate(range(0, len(waits), MAX_WAITS_PER_INST)):
                nop = engs[j % len(engs)].nop()
                nop.ins.sync_info = mybir.SyncInfo(
                    on_wait=waits[i : i + MAX_WAITS_PER_INST], on_update=[]
                )
        nc.all_engine_barrier()
        assert self.sems is not None
        popped = nc._tile_sem_poison_stack.pop()
        assert popped is self._sem_poison
        nc.clear_and_free_semaphores(list(self.sems.allocated().values()))
        nc.all_engine_barrier()


def build_nc() -> bass.Bass:
    nc = bass.Bass()
    ctx_d = nc.dram_tensor(
        "ctx", [B_PER_CORE, 128, N_CTILES, H], F16, kind="ExternalInput"
    )
    ctxT_d = nc.dram_tensor(
        "ctxT", [B_PER_CORE, H, C_LEN], F16, kind="ExternalInput"
    )
    blob_d = nc.dram_tensor(
        "blob", [B_PER_CORE, 128, BLOB_COLS], F16, kind="ExternalInput"
    )
    g_d = nc.dram_tensor(
        "G", [B_PER_CORE, 128, N_CTILES, 384], F16, kind="ExternalOutput"
    )

    from contextlib import ExitStack

    with SplitDrainTileContext(nc) as tc, ExitStack() as es:
        consts = es.enter_context(tc.tile_pool(name="consts", bufs=1))
        batchp = es.enter_context(tc.tile_pool(name="batchp", bufs=2))
        ptt = es.enter_context(tc.tile_pool(name="ptt", bufs=2, space="PSUM"))
        pte = es.enter_context(tc.tile_pool(name="pte", bufs=2, space="PSUM"))
        pcq = es.enter_context(tc.tile_pool(name="pcq", bufs=2, space="PSUM"))
        pmisc = es.enter_context(tc.tile_pool(name="pmisc", bufs=2, space="PSUM"))
        # pmisc layout (one [128, 178] f32 tile per batch):
        #   cols 0:16  s_c       col 16     s_q
        #   cols 17:33 Z cols    row0 34:162 q2c row   row0 162:178 zb
        #   cols 178:306 bc (q2c broadcast)

        identity = consts.tile([128, 128], F16)
        make_identity(nc, identity)
        ones_col = consts.tile([128, 1], F16)
        nc.vector.memset(ones_col, 1.0)
        ones_row = consts.tile([1, 128], F16)
        nc.vector.memset(ones_row, 1.0)

        def emit_loads(b):
            L = {}
            blob = batchp.tile([128, BLOB_COLS], F16, tag="blob")
            nc.sync.dma_start(out=blob, in_=blob_d[b])
            # ctxT in per-chunk pieces so the first S^T matmul starts early
            ctxT = batchp.tile([128, C_LEN], F16, tag="ctxT")
            for g in range(NGRP):
                c0 = g * GS * 128
                nc.sync.dma_start(
                    out=ctxT[:, c0 : c0 + GS * 128],
                    in_=ctxT_d[b][:, c0 : c0 + GS * 128],
                )
            ctx = batchp.tile([128, N_CTILES, H], F16, tag="ctx")
            nc.sync.dma_start(out=ctx, in_=ctx_d[b])
            L.update(blob=blob, ctxT=ctxT, ctx=ctx)
            return L

        def emit_prelims(b, L):
            blob, ctxT = L["blob"], L["ctxT"]
            qT = blob[:, BC_QT : BC_QT + 128]
            w_cq = blob[:, BC_WCQ : BC_WCQ + 1]
            w_q = blob[:, BC_WQ : BC_WQ + 1]
            w_c = blob[:, BC_WC : BC_WC + 1]

            # qTw[h, q] = queryT * w_cq  (stationary operand of the S^T matmul)
            w_cq32 = batchp.tile([128, 1], F32, tag="w_cq32")
            nc.vector.tensor_copy(out=w_cq32, in_=w_cq)
            qTw = batchp.tile([128, 128], F16, tag="qTw")
            nc.vector.tensor_scalar_mul(qTw, qT, w_cq32)

            misc = pmisc.tile([128, 320], F32, tag="misc")
            L["misc"] = misc

            # s_q[q] + qbias -> sqm_col (exp bias)
            nc.tensor.matmul(misc[:, 16:17], qT, w_q, start=True, stop=True)
            sqm_col = batchp.tile([128, 1], F32, tag="sqm_col")
            nc.gpsimd.tensor_add(
                out=sqm_col, in0=misc[:, 16:17], in1=blob[:, BC_QB : BC_QB + 1]
            )

            # s_c[(p, t)] via 16 tiny matmuls, then exp(s_c + cbias) -> ebp
            ps_sc = misc[:, 0:N_CTILES]
            for t in range(N_CTILES):
                nc.tensor.matmul(
                    ps_sc[:, t : t + 1],
                    ctxT[:, t * 128 : (t + 1) * 128],
                    w_c,
                    start=True,
                    stop=True,
                )
            upre = batchp.tile([128, N_CTILES], F32, tag="upre")
            nc.gpsimd.tensor_add(
                out=upre, in0=ps_sc, in1=blob[:, BC_CB : BC_CB + N_CTILES]
            )
            ebp = batchp.tile([128, N_CTILES], F32, tag="ebp")
            nc.scalar.activation(
                out=ebp, in_=upre, func=mybir.ActivationFunctionType.Exp
            )

            eT = batchp.tile([128, C_LEN], F16, tag="eT")
            m_buf = batchp.tile([128, N_CTILES], F32, tag="m_buf")
            eb16 = batchp.tile([128, N_CTILES], F16, tag="eb16")
            gbuf = batchp.tile([128, N_CTILES, 384], F16, tag="gbuf")
            L.update(
                qTw=qTw, sqm_col=sqm_col, ebp=ebp, eT=eT, m_buf=m_buf,
                eb16=eb16, gbuf=gbuf,
            )

        def emit_group(b, L, g):
            """S^T chunk -> exp -> rowmax; then c2q/G2/G3 for the group."""
            blob, ctxT, ctx = L["blob"], L["ctxT"], L["ctx"]
            qTw, sqm_col, eT = L["qTw"], L["sqm_col"], L["eT"]
            m_buf, gbuf = L["m_buf"], L["gbuf"]
            c0 = g * GS * 128
            cw = GS * 128

            # S^T chunk: [q, 512] = qTw^T-contracted against ctxT cols
            ps_tt = ptt.tile([128, cw], F32, tag="tt")
            nc.tensor.matmul(
                ps_tt, qTw, ctxT[:, c0 : c0 + cw], start=True, stop=True
            )
            nc.scalar.activation(
                out=eT[:, c0 : c0 + cw],
                in_=ps_tt,
                func=mybir.ActivationFunctionType.Exp,
                bias=sqm_col,
            )

            # rowmax over q (via PE transpose): m~[c] = max_q eT[q, c]
            ps_te = pte.tile([128, GS, 128], F16, tag="te")
            for i in range(GS):
                t = g * GS + i
                nc.tensor.transpose(
                    ps_te[:, i, :], eT[:, t * 128 : (t + 1) * 128], identity
                )
            nc.vector.tensor_reduce(
                out=m_buf[:, g * GS : (g + 1) * GS],
                in_=ps_te,
                axis=mybir.AxisListType.X,
                op=mybir.AluOpType.max,
            )

            # c2q numerators + Z cols for the group
            misc = L["misc"]
            ps_cq = pcq.tile([128, GS, 128], F32, tag="cq")
            ps_z = misc[:, 17 + g * GS : 17 + (g + 1) * GS]
            for i in range(GS):
                t = g * GS + i
                nc.tensor.matmul(
                    ps_cq[:, i, :],
                    eT[:, t * 128 : (t + 1) * 128],
                    blob[:, BC_Q : BC_Q + 128],
                    start=True,
                    stop=True,
                )
                nc.tensor.matmul(
                    ps_z[:, i : i + 1],
                    eT[:, t * 128 : (t + 1) * 128],
                    ones_col,
                    start=True,
                    stop=True,
                )
            # G2 = c2q / Z: broadcast Z over the h dim, divide on gpsimd
            gs = slice(g * GS, (g + 1) * GS)
            z_bc = ps_z.unsqueeze(2).broadcast_to([128, GS, 128])
            nc.gpsimd.tensor_tensor(
                out=gbuf[:, gs, 0:128], in0=ps_cq, in1=z_bc,
                op=mybir.AluOpType.divide,
            )
            # G3 = ctx * c2q (fp16, 2x DVE)
            nc.vector.tensor_mul(
                out=gbuf[:, gs, 128:256],
                in0=gbuf[:, gs, 0:128],
                in1=ctx[:, gs, :],
            )

        def emit_q2c_tail(b, L):
            """After all rowmaxes: e_b, q2c row, normalize, broadcast."""
            misc, gbuf, ctx, eb16 = L["misc"], L["gbuf"], L["ctx"], L["eb16"]
            nc.vector.tensor_mul(out=eb16, in0=L["ebp"], in1=L["m_buf"])
            ps_q2c = misc[0:1, 34:162]
            for t in range(N_CTILES):
                nc.tensor.matmul(
                    ps_q2c,
                    eb16[:, t : t + 1],
                    ctx[:, t, :],
                    start=(t == 0),
                    stop=(t == N_CTILES - 1),
                )
            ps_zb = misc[0:1, 162:178]
            nc.tensor.matmul(ps_zb, ones_col, eb16, start=True, stop=True)
            z_tot = batchp.tile([1, 1], F32, tag="z_tot")
            nc.vector.reduce_sum(
                out=z_tot, in_=ps_zb, axis=mybir.AxisListType.X
            )
            zr = batchp.tile([1, 1], F32, tag="zr")
            nc.vector.reciprocal(out=zr, in_=z_tot)
            # zr_row = zr broadcast along the row: folds the 1/Z into the
            # bc matmul's stationary operand (bc = zr_row^T . q2cc16)
            zr_row = batchp.tile([1, 128], F16, tag="zr_row")
            nc.vector.tensor_scalar_mul(zr_row, ones_row, zr)
            # unnormalized q2c row -> fp16 (no zr dependency; runs on Act)
            q2cc = batchp.tile([1, 128], F16, tag="q2cc")
            nc.scalar.copy(out=q2cc, in_=ps_q2c)

            ps_bc = misc[:, 178:306]
            nc.tensor.matmul(ps_bc, zr_row, q2cc, start=True, stop=True)
            bc_sb = batchp.tile([128, 128], F16, tag="bc_sb")
            nc.scalar.copy(out=bc_sb, in_=ps_bc)
            L["bc_sb"] = bc_sb

        def emit_g4_store(b, L, g):
            ctx, gbuf, bc_sb = L["ctx"], L["gbuf"], L["bc_sb"]
            gs = slice(g * GS, (g + 1) * GS)
            # G4 = ctx * q2c (broadcast over t)
            bc_bc = bc_sb.unsqueeze(1).broadcast_to([128, GS, 128])
            nc.vector.tensor_mul(
                out=gbuf[:, gs, 256:384], in0=ctx[:, gs, :], in1=bc_bc
            )
            nc.sync.dma_start(out=g_d[b][:, gs, :], in_=gbuf[:, gs, :])

        # ---- emission schedule: 2 batches, software-pipelined ----
        Ls = [emit_loads(b) for b in range(B_PER_CORE)]
        emit_prelims(0, Ls[0])
        for g in range(NGRP):
            emit_group(0, Ls[0], g)
        emit_prelims(1, Ls[1])
        emit_q2c_tail(0, Ls[0])
        emit_group(1, Ls[1], 0)
        emit_g4_store(0, Ls[0], 0)
        emit_group(1, Ls[1], 1)
        emit_g4_store(0, Ls[0], 1)
        emit_group(1, Ls[1], 2)
        emit_g4_store(0, Ls[0], 2)
        emit_group(1, Ls[1], 3)
        emit_g4_store(0, Ls[0], 3)
        emit_q2c_tail(1, Ls[1])
        for g in range(NGRP):
            emit_g4_store(1, Ls[1], g)

    return nc


# ---------------- host-side packing ----------------

def pack_core_inputs(context, query, W, context_mask, query_mask, core):
    """Build the per-core device input map (fp16-packed)."""
    sl = slice(core * B_PER_CORE, (core + 1) * B_PER_CORE)
    ctx = context[sl]                      # [2, 2048, 128] f32
    qry = query[sl]                        # [2, 128, 128] f32
    cm = context_mask[sl]                  # [2, 2048] i32
    qm = query_mask[sl]                    # [2, 128] i32

    ctx16 = ctx.astype(np.float16)
    # [2, 2048, 128] -> [2, 16, 128, 128] -> [2, 128(p), 16(t), 128(h)]
    ctx_p = np.ascontiguousarray(
        ctx16.reshape(B_PER_CORE, N_CTILES, 128, H).transpose(0, 2, 1, 3)
    )
    ctxT_p = np.ascontiguousarray(ctx16.transpose(0, 2, 1))  # [2, 128h, 2048c]

    blob = np.zeros((B_PER_CORE, 128, BLOB_COLS), dtype=np.float16)
    q16 = qry.astype(np.float16)
    blob[:, :, BC_QT : BC_QT + 128] = q16.transpose(0, 2, 1)
    blob[:, :, BC_Q : BC_Q + 128] = q16
    # bias = (mask-1)*30000: 0 where mask==1, -30000 (exp->0) where mask==0
    blob[:, :, BC_QB] = ((qm.astype(np.float32) - 1.0) * -MASK_NEG).astype(np.float16)
    cb = (cm.astype(np.float32) - 1.0) * -MASK_NEG
    blob[:, :, BC_CB : BC_CB + N_CTILES] = (
        cb.reshape(B_PER_CORE, N_CTILES, 128).transpose(0, 2, 1).astype(np.float16)
    )
    w_c, w_q, w_cq = W[:H], W[H : 2 * H], W[2 * H :]
    blob[:, :, BC_WC] = w_c.astype(np.float16)[None, :]
    blob[:, :, BC_WQ] = w_q.astype(np.float16)[None, :]
    blob[:, :, BC_WCQ] = w_cq.astype(np.float16)[None, :]
    return {"ctx": ctx_p, "ctxT": ctxT_p, "blob": blob}


def unpack_core_output(g_dev):
    """[2, 128p, 16t, 384] fp16 -> [2, 2048, 384] f32."""
    return (
        g_dev.transpose(0, 2, 1, 3)
        .reshape(B_PER_CORE, C_LEN, 384)
        .astype(np.float32)
    )


_NC_CACHE = None


def _get_nc():
    global _NC_CACHE
    if _NC_CACHE is None:
        _NC_CACHE = build_nc()
    return _NC_CACHE


def kernel(context, query, W, context_mask, query_mask):
    from concourse.bass_utils import run_bass_kernel_spmd

    context = np.ascontiguousarray(np.asarray(context, dtype=np.float32))
    query = np.ascontiguousarray(np.asarray(query, dtype=np.float32))
    W = np.ascontiguousarray(np.asarray(W, dtype=np.float32))
    context_mask = np.ascontiguousarray(np.asarray(context_mask, dtype=np.int32))
    query_mask = np.ascontiguousarray(np.asarray(query_mask, dtype=np.int32))

    nc = _get_nc()
    in_maps = [
        pack_core_inputs(context, query, W, context_mask, query_mask, c)
        for c in range(N_CORES)
    ]
    res = run_bass_kernel_spmd(nc, in_maps, core_ids=list(range(N_CORES)))

    out = np.empty((B, C_LEN, 4 * H), dtype=np.float32)
    out[:, :, 0:128] = context  # G block 0 is the context passthrough
    for c in range(N_CORES):
        sl = slice(c * B_PER_CORE, (c + 1) * B_PER_CORE)
        out[sl, :, 128:512] = unpack_core_output(np.asarray(res.results[c]["G"]))
    return out


if __name__ == "__main__":
    from concourse.timeline_sim import TimelineSim

    nc = build_nc()
    dur = TimelineSim(nc).simulate()
    print(f"TimelineSim estimated duration: {dur:.0f} ns")
